# revision 24
# baseline (speedup 1.0000x reference)
"""Trainium2 Bass kernel v3: constant-geometry data-dependent FFT.

Structure per core (512 features = 4 groups x 128 partitions; features on
partitions, FFT points along the free dim, packed complex fp16 granules):

  - Host sends x fp16 with rows pre-permuted into quad layout, and a
    phase image PH holding every twiddle phase (range-reduced, incl. the
    cos phases as 0.25-|r|) laid out exactly like the on-chip pack.
  - ONE Sin activation turns PH into the complete twiddle pack (fp16
    (cos,sin) granules, stage s at el offset 2^s).
  - Stages 1+2: 8 custom 2X DVE ops over row quads -> T3/B3.
  - Stages 3..12: per stage:
      CMUL (custom 2X, 2-tensor):  U = pack_s (*) B
      combineT/combineB: hijacked stock TENSOR_SCALAR row running a
      4X_2PORT butterfly uop program (2 granules/cycle): reads the
      (T half || U half) 2-run AP, writes y0/y1 interleaved into the
      next stage's T (or B) region.
  - Output O (fp16 packed complex) DMA'd out in 2 chunks; host converts
    to complex64 and un-permutes rows (bit-reversal-like final_perm).

Everything validated against the reference network in sim_plan3.py (exact)
and perf modes/port semantics probed on hardware (bench_dve/bench_hijack*).
"""

import math
import sys

import numpy as np

if "/opt/trn_rl_repo" not in sys.path:
    sys.path.insert(0, "/opt/trn_rl_repo")

import concourse.bacc as bacc
import concourse.bass_isa as bass_isa
import concourse.mybir as mybir
from concourse.bass_utils import run_bass_kernel_spmd
from concourse.tile import TileContext

import concourse.dve_ops as dve_ops
from concourse.dve_spec import Spec, Src0, Src1
from concourse.dve_uop import (
    AluInp,
    AluOp,
    DelayInp,
    DveOpSpec,
    InpSel,
    OutPath,
    OutSel,
    Trigger,
    UopConfig,
)

F32 = mybir.dt.float32
F16 = mybir.dt.float16
AF = mybir.ActivationFunctionType

N = 4096
LOGN = 12
K = N // 2  # butterflies per stage
NCORES = 8
DSH = N // NCORES
NGROUPS = DSH // 128
TWO_PI = 2.0 * math.pi

D = [
    AluInp.PREV_DELAY_0,
    AluInp.PREV_DELAY_1,
    AluInp.PREV_DELAY_2,
    AluInp.PREV_DELAY_3,
    AluInp.PREV_DELAY_4,
    AluInp.PREV_DELAY_5,
]


def _uop(inputs, req0, req1, trigger, next_uop, repeat=0):
    u = UopConfig()
    for lane, sel in enumerate(inputs, start=1):
        u.enable_input(sel, lane)
    u.require_inp0 = req0
    u.require_inp1 = req1
    u.trigger = trigger
    u.next_uop = next_uop
    u.repeat_count = repeat
    return u


_TRIG1 = (Trigger.SRC_TENSOR_DONE, Trigger.NONE, Trigger.NONE)


# ===================== custom 2X ops (CMUL + stage-1/2 quads) ==============


def _cmul_uop():
    # packed complex multiply: out = in0 (*) in1, 1 granule/cycle in 2X
    u = _uop(
        [InpSel.SRC_0, InpSel.SRC_1, InpSel.SRC_0_HI, InpSel.SRC_1_HI],
        1, 1, _TRIG1, (0, 0, 0),
    )
    dp = u.datapath_config
    dp[0].enable_alu(AluOp.MULTIPLY, D[0], D[1])
    dp[0].pass_through_delay(0, 1, 2, 3)
    dp[1].enable_alu(AluOp.MULTIPLY, D[2], D[3])
    dp[1].pass_through_delay(0, 1, 2, 3)
    dp[1].enable_delay_from_src(DelayInp.PREV_ALU_OUT, 4)
    dp[2].enable_alu(AluOp.SUBTRACT, D[4], AluInp.PREV_ALU_OUT)
    dp[2].pass_through_delay(0, 1, 2, 3)
    dp[3].enable_alu(AluOp.MULTIPLY, D[0], D[3])
    dp[3].pass_through_delay(1, 2)
    dp[3].enable_delay_from_src(DelayInp.PREV_ALU_OUT, 4)
    dp[4].enable_alu(AluOp.MULTIPLY, D[2], D[1])
    dp[4].pass_through_delay(4)
    dp[4].enable_delay_from_src(DelayInp.PREV_ALU_OUT, 0)
    dp[5].enable_alu(AluOp.ADD, D[0], AluInp.PREV_ALU_OUT)
    dp[5].pass_through_delay(4)
    dp[6].pass_through_alu()
    dp[6].pass_through_delay(4)
    dp[7].pass_through_alu()
    dp[7].pass_through_delay(4)
    u.enable_output(OutSel.DELAY_4, OutPath.WR0_LO)
    u.enable_output(OutSel.ALU_OUT, OutPath.WR0_HI)
    return u


def _cmul_reference(in0, in1, c0, c1, c2):
    a = in0.astype(np.float32)
    b = np.broadcast_to(in1, in0.shape).astype(np.float32)
    out = np.empty_like(a)
    ar, ai = a[..., 0::2], a[..., 1::2]
    br, bi = b[..., 0::2], b[..., 1::2]
    out[..., 0::2] = ar * br - ai * bi
    out[..., 1::2] = ar * bi + ai * br
    return out


# stage-1/2 quad ops: per cycle granule A=(a,b), granule B=(c,d); one
# output granule. t0=a+b t1=c+d t2=a-b u=c-d.
#  S3K0 -> (t0+t1, 0)      S3K2 -> (t0-t1, 0)
#  S3K1 -> (t2 + C0*u, C1*u)   [s0=cos col, s1=sin col]
#  S3K3 -> (t2 - C0*u, C1*u)   [s0=cos col, s1=-sin col]


def _s3k_sum(sub: bool):
    u = _uop(
        [InpSel.SRC_0, InpSel.SRC_0_HI, InpSel.SRC_1, InpSel.SRC_1_HI,
         InpSel.ZERO],
        1, 1, _TRIG1, (0, 0, 0),
    )
    dp = u.datapath_config
    dp[0].enable_alu(AluOp.ADD, D[0], D[1])  # t0
    dp[0].pass_through_delay(2, 3, 4)
    dp[1].enable_alu(AluOp.ADD, D[2], D[3])  # t1
    dp[1].enable_delay_from_src(DelayInp.PREV_ALU_OUT, 0)  # t0
    dp[1].pass_through_delay(2, 3, 4)
    op = AluOp.SUBTRACT if sub else AluOp.ADD
    dp[2].enable_alu(op, D[0], AluInp.PREV_ALU_OUT)  # re = t0 +/- t1
    dp[2].pass_through_delay(4)
    for k in range(3, 8):
        dp[k].pass_through_alu()
        dp[k].pass_through_delay(4)
    u.enable_output(OutSel.ALU_OUT, OutPath.WR0_LO)  # re
    u.enable_output(OutSel.DELAY_4, OutPath.WR0_HI)  # 0
    return u


def _s3k_tw(sub: bool):
    u = _uop(
        [InpSel.SRC_0, InpSel.SRC_0_HI, InpSel.SRC_1, InpSel.SRC_1_HI,
         InpSel.CONST_0, InpSel.CONST_1],
        1, 1, _TRIG1, (0, 0, 0),
    )
    dp = u.datapath_config
    dp[0].enable_alu(AluOp.SUBTRACT, D[0], D[1])  # t2 = a-b
    dp[0].pass_through_delay(2, 3, 4, 5)
    dp[1].enable_alu(AluOp.SUBTRACT, D[2], D[3])  # u = c-d
    dp[1].enable_delay_from_src(DelayInp.PREV_ALU_OUT, 0)  # t2
    dp[1].pass_through_delay(4, 5)
    dp[2].enable_alu(AluOp.MULTIPLY, AluInp.PREV_ALU_OUT, D[4])  # mc = C0*u
    dp[2].enable_delay_from_src(DelayInp.PREV_ALU_OUT, 1)  # u
    dp[2].pass_through_delay(0, 5)
    op = AluOp.SUBTRACT if sub else AluOp.ADD
    dp[3].enable_alu(op, D[0], AluInp.PREV_ALU_OUT)  # re = t2 +/- mc
    dp[3].pass_through_delay(1, 5)
    dp[4].enable_alu(AluOp.MULTIPLY, D[1], D[5])  # im = C1*u
    dp[4].enable_delay_from_src(DelayInp.PREV_ALU_OUT, 0)  # re
    for k in range(5, 8):
        dp[k].pass_through_alu()
        dp[k].pass_through_delay(0)
    u.enable_output(OutSel.DELAY_0, OutPath.WR0_LO)  # re
    u.enable_output(OutSel.ALU_OUT, OutPath.WR0_HI)  # im
    return u


def _s3k_sum_ref(sub):
    def f(in0, in1, c0, c1, c2):
        a = in0.astype(np.float32)
        b = np.broadcast_to(in1, in0.shape).astype(np.float32)
        t0 = a[..., 0::2] + a[..., 1::2]
        t1 = b[..., 0::2] + b[..., 1::2]
        out = np.empty_like(a)
        out[..., 0::2] = t0 - t1 if sub else t0 + t1
        out[..., 1::2] = 0.0
        return out

    return f


def _s3k_tw_ref(sub):
    def f(in0, in1, c0, c1, c2):
        a = in0.astype(np.float32)
        b = np.broadcast_to(in1, in0.shape).astype(np.float32)
        c0 = np.asarray(c0, np.float32).reshape(-1, *([1] * (a.ndim - 1)))
        c1 = np.asarray(c1, np.float32).reshape(-1, *([1] * (a.ndim - 1)))
        t2 = a[..., 0::2] - a[..., 1::2]
        u = b[..., 0::2] - b[..., 1::2]
        out = np.empty_like(a)
        out[..., 0::2] = t2 - c0 * u if sub else t2 + c0 * u
        out[..., 1::2] = c1 * u
        return out

    return f


# ===================== 4X combine on the hijacked stock row ================

TS_ROW_OPCODES = (67, 68)  # TENSOR_SCALAR family rows (imm + ptr variants)


def _bfly4x():
    # half-split ports: A granule from in-AP first half, B from second half;
    # WR0 (first half of out walk) = A+B, WR1 (second half) = A-B.
    u = _uop(
        [InpSel.SRC_0, InpSel.SRC_0_HI, InpSel.SRC_1, InpSel.SRC_1_HI],
        1, 1, _TRIG1, (0, 0, 0),
    )
    u.enable_input(InpSel.SRC_0, 0)
    dp = u.datapath_config
    dp[0].enable_alu(AluOp.ADD, D[0], D[2])
    dp[0].pass_through_delay(0, 1, 2, 3)
    dp[1].enable_alu(AluOp.ADD, D[1], D[3])
    dp[1].enable_delay_from_src(DelayInp.PREV_ALU_OUT, 4)
    dp[1].pass_through_delay(0, 1, 2, 3)
    dp[2].enable_alu(AluOp.SUBTRACT, D[0], D[2])
    dp[2].enable_delay_from_src(DelayInp.PREV_ALU_OUT, 5)
    dp[2].pass_through_delay(1, 3, 4)
    dp[3].enable_alu(AluOp.SUBTRACT, D[1], D[3])
    dp[3].enable_delay_from_src(DelayInp.PREV_ALU_OUT, 0)
    dp[3].pass_through_delay(4, 5)
    for k in range(4, 8):
        dp[k].pass_through_alu()
        dp[k].pass_through_delay(0, 4, 5)
    u.enable_output(OutSel.DELAY_4, OutPath.WR0_LO)
    u.enable_output(OutSel.DELAY_5, OutPath.WR0_HI)
    u.enable_output(OutSel.DELAY_0, OutPath.WR1_LO)
    u.enable_output(OutSel.ALU_OUT, OutPath.WR1_HI)
    return u


def _bfly_ref(in0, in1, c0, c1, c2):
    return in0.astype(np.float32)


# ===================== registry ============================================


class StockRowSpec(DveOpSpec):
    """Spec for a hijacked STOCK row — skip the custom_dve-only uop guards."""

    def validate(self, ver):
        for uops in (self.uops, self.uops_2x, self.uops_2x_2p, self.uops_4x):
            if uops is None:
                continue
            for u in uops:
                for ni in u.next_uop:
                    assert ni < len(uops)


class RawDveOp:
    def __init__(self, name, mk_uops, rd1_en, perf_max, reference,
                 opcode=None, all_modes=False):
        self.name = name
        self.subdim = False
        self.spec = Spec(body=Src0 * Src1 if rd1_en else Src0,
                         reference=reference)
        self.rd1_en = rd1_en
        self.perf_max = perf_max
        self.opcode = opcode
        self.all_modes = all_modes
        self._mk = mk_uops
        self._cache = {}

    def compile(self, ver):
        if ver in self._cache:
            return self._cache[ver]
        cls = StockRowSpec if self.opcode is not None else DveOpSpec
        spec = cls(
            name=self.name,
            uops=[self._mk()],
            opcode=self.opcode
            if self.opcode is not None
            else dve_ops.get_dve_sub_opcode(self.name),
            uops_2x=[self._mk()],
            uops_2x_2p=[self._mk()] if self.all_modes else None,
            uops_4x=[self._mk()] if self.all_modes else None,
            perf_max=self.perf_max,
            rd1_en=self.rd1_en,
        )
        spec.validate(ver)
        self._cache[ver] = spec
        return spec


RAW3 = {}


def register_raw_ops3():
    if RAW3:
        return RAW3
    defs = [
        RawDveOp("CMUL3_ANT", _cmul_uop, True, 1, _cmul_reference),
        RawDveOp("S3K0_ANT", lambda: _s3k_sum(False), True, 1, _s3k_sum_ref(False)),
        RawDveOp("S3K2_ANT", lambda: _s3k_sum(True), True, 1, _s3k_sum_ref(True)),
        RawDveOp("S3K1_ANT", lambda: _s3k_tw(False), True, 1, _s3k_tw_ref(False)),
        RawDveOp("S3K3_ANT", lambda: _s3k_tw(True), True, 1, _s3k_tw_ref(True)),
        RawDveOp("BFLY43_HJ", _bfly4x, False, 0, _bfly_ref, opcode=67,
                 all_modes=True),
        RawDveOp("BFLY44_HJ", _bfly4x, False, 0, _bfly_ref, opcode=68,
                 all_modes=True),
    ]
    for op in defs:
        if op.name not in dve_ops._SUB_OPCODE_FOR_NAME:
            dve_ops.OPS.append(op)
            if op.opcode is None:
                row = dve_ops._CUSTOM_DVE_ROW_BASE + len(dve_ops.OPS) - 1
                assert row < 0x20, row
                dve_ops._SUB_OPCODE_FOR_NAME[op.name] = row
            else:
                dve_ops._SUB_OPCODE_FOR_NAME[op.name] = op.opcode
            dve_ops.CUSTOM_DVE_SPECS[op.name] = op.spec
        RAW3[op.name] = op
    return RAW3


def emit_raw3(nc, name, out, in0, in1=None, s0=None, s1=None, perf_max=1,
              force_ttss=False):
    ops = register_raw_ops3()
    op = ops[name]
    v = nc.vector
    if op.name not in nc.m.ant_custom_dve_ops:
        nc.m.ant_custom_dve_ops = sorted({*nc.m.ant_custom_dve_ops, op.name})
    use_stt = in1 is not None and not force_ttss
    shape = bass_isa.CustomDveShape.STT if use_stt else bass_isa.CustomDveShape.TTSS
    isa_opcode = nc.isa.Opcode[
        f"NEURON_ISA_TPB_OPCODE_CUSTOM_DVE_ANT_{shape.slot()}"
    ].value
    imm = mybir.ImmediateValue(dtype=mybir.dt.float32, value=0.0)
    s0a = v.lower_ap(s0, for_isa=True) if s0 is not None else imm
    s1a = v.lower_ap(s1, for_isa=True) if s1 is not None else imm
    ins = [v.lower_ap(in0, for_isa=True)]
    if in1 is not None:
        ins.append(v.lower_ap(in1, for_isa=True))
    ins += [s0a, s1a]
    return v.add_instruction(
        bass_isa.InstCustomDveAnt(
            name=nc.get_next_instruction_name(),
            op_name=op.name,
            rd1_en=op.rd1_en,
            subdim=0,
            imm2=0.0,
            shape=shape,
            row=dve_ops.get_dve_sub_opcode(op.name),
            isa_opcode=isa_opcode,
            perf_max=perf_max,
            ins=ins,
            outs=[v.lower_ap(out, for_isa=True)],
        )
    )


# ===================== kernel builder ======================================


def build_fft_nc():
    register_raw_ops3()
    nc = bacc.Bacc()
    # hijacked stock rows must land in the per-NEFF DVE table
    nc.m.ant_custom_dve_ops = sorted(
        {*nc.m.ant_custom_dve_ops, "BFLY43_HJ", "BFLY44_HJ"}
    )

    xT = nc.dram_tensor("xT", [DSH, N], F16, kind="ExternalInput")
    phT = nc.dram_tensor("phT", [DSH, 2 * N], F16, kind="ExternalInput")
    outT = nc.dram_tensor("outT", [DSH, 2 * N], F16, kind="ExternalOutput")

    EL_T = 0          # T region: els [0, 4096)
    EL_B = N          # B region: els [4096, 8192)  (adjacent to T!)
    EL_U = 2 * N      # U region: els [8192, 12288)

    with TileContext(nc) as tc:
        with (
            tc.tile_pool(name="xin", bufs=2) as xpool,
            tc.tile_pool(name="pk", bufs=2) as kpool,
            tc.tile_pool(name="tub", bufs=2) as tpool,
            tc.tile_pool(name="stg", bufs=2) as spool,
            tc.tile_pool(name="obuf", bufs=1) as opool,
        ):
            packs = {}
            c2s = {}
            xrs = {}
            stags = {}
            tubss = {}

            def load_x(g):
                # Queue discipline: gpsimd carries only tiny latency-critical
                # transfers (ph heads, group-0 x halves); the scalar HWDGE
                # queue carries dependency-free bulk input (its triggers fire
                # immediately, so they never block later scalar-engine work);
                # sync carries group-0 x halves and all output stores.
                rows = slice(g * 128, (g + 1) * 128)
                xr = xpool.tile([128, N], F16, tag="xr")
                xrs[g] = xr
                if g == 0:
                    # group 0's x is latency-critical: stripe the four
                    # quarter-chunks across both queues, par-0 cols first
                    nc.sync.dma_start(xr[:, 0:1024], xT[rows, 0:1024])
                    nc.gpsimd.dma_start(xr[:, 2048:3072], xT[rows, 2048:3072])
                    nc.sync.dma_start(xr[:, 1024:2048], xT[rows, 1024:2048])
                    nc.gpsimd.dma_start(xr[:, 3072:4096], xT[rows, 3072:4096])
                else:
                    nc.scalar.dma_start(xr[:], xT[rows, :])

            def load_ph(g):
                rows = slice(g * 128, (g + 1) * 128)
                # pack doubles as the ph landing buffer: Sin runs in place.
                pack = kpool.tile([128, 2 * N], F16, tag="pack")
                c2 = kpool.tile([128, 4], F32, tag="c2")
                packs[g], c2s[g] = pack, c2
                # ph head on gpsimd: lands fast, c2 is on the quads' critical
                # path. mid covers stage 7..9 twiddles, rest stages 10..12.
                nc.gpsimd.dma_start(pack[:, 0:128], phT[rows, 0:128])
                nc.scalar.dma_start(pack[:, 128:2048], phT[rows, 128:2048])
                nc.scalar.dma_start(pack[:, 2048 : 2 * N], phT[rows, 2048 : 2 * N])
                nc.scalar.activation(c2[:, 0:3], pack[:, 0:3], AF.Sin,
                                     scale=TWO_PI)
                nc.scalar.activation(pack[:, 0:128], pack[:, 0:128], AF.Sin,
                                     scale=TWO_PI)

            def sins_rest(g):
                pack = packs[g]
                nc.scalar.activation(pack[:, 128:2048], pack[:, 128:2048],
                                     AF.Sin, scale=TWO_PI)
                nc.scalar.activation(pack[:, 2048 : 2 * N], pack[:, 2048 : 2 * N],
                                     AF.Sin, scale=TWO_PI)

            def quads_group(g):
                # stages 1+2: 8 quad ops (2X, contiguous outs into staging)
                xr, c2 = xrs[g], c2s[g]
                stag = spool.tile([128, 2 * N], F16, tag="stg")
                stags[g] = stag
                tubA = tpool.tile([128, 3 * N], F16, tag="tubA")
                tubB = tpool.tile([128, 3 * N], F16, tag="tubB")
                tubss[g] = [tubA, tubB]
                for par in (0, 1):
                    ab = xr[:, par * 1024 : par * 1024 + 1024]
                    cd = xr[:, 2048 + par * 1024 : 2048 + par * 1024 + 1024]
                    sb = par * N  # staging region per parity: k-major blocks
                    emit_raw3(nc, "S3K0_ANT",
                              stag[:, sb + 0 : sb + 1024], ab, cd)
                    emit_raw3(nc, "S3K1_ANT",
                              stag[:, sb + 1024 : sb + 2048], ab, cd,
                              s0=c2[:, 0:1], s1=c2[:, 1:2], force_ttss=True)
                    emit_raw3(nc, "S3K2_ANT",
                              stag[:, sb + 2048 : sb + 3072], ab, cd)
                    emit_raw3(nc, "S3K3_ANT",
                              stag[:, sb + 3072 : sb + 4096], ab, cd,
                              s0=c2[:, 0:1], s1=c2[:, 2:3], force_ttss=True)

            def copies_group(g):
                # scatter quad-order staging into pair-order T3/B3 on the
                # (otherwise idle) scalar engine; dst granule 4r+k <- src
                # granule k*512+r
                stag, cur = stags[g], tubss[g][0]
                for par in (0, 1):
                    sb = par * N
                    base = EL_T if par == 0 else EL_B
                    r4 = (
                        cur[:, base : base + N]
                        .rearrange("p (r k f) -> p k r f", k=4, f=2)
                    )
                    # group 0's copies are on the critical path (nothing else
                    # for the DVE to chew on), so the DVE itself scatters the
                    # par-1 (B3) half it needs first, while ACT does par-0
                    eng = nc.vector if (g == 0 and par == 1) else nc.scalar
                    if eng is nc.vector:
                        eng.tensor_copy(r4[:, 0:2], stag[:, sb : sb + 2048])
                        eng.tensor_copy(r4[:, 2:4], stag[:, sb + 2048 : sb + 4096])
                    else:
                        eng.copy(r4[:, 0:2], stag[:, sb : sb + 2048])
                        eng.copy(r4[:, 2:4], stag[:, sb + 2048 : sb + 4096])

            def one_stage(g, si, s, obuf):
                rows = slice(g * 128, (g + 1) * 128)
                pack, tubs = packs[g], tubss[g]
                cur = tubs[si % 2]
                nxt = tubs[(si + 1) % 2]
                half = 1 << (s - 1)
                rep = N >> s
                twp = pack[:, 2 * half : 4 * half]
                if rep > 1:
                    tw = twp.unsqueeze(1).broadcast_to([128, rep, 2 * half])
                else:
                    tw = twp
                emit_raw3(
                    nc, "CMUL3_ANT",
                    cur[:, EL_U : EL_U + N],
                    cur[:, EL_B : EL_B + N],
                    tw,
                )
                if s < LOGN:
                    # one combine: in = (all T || all U) 2-run AP; out =
                    # y0/y1 interleaved over the adjacent T|B dest region
                    src = (
                        cur[:, 0 : 3 * N]
                        .rearrange("p (h f) -> p h f", h=3)[:, 0:3:2]
                    )
                    dst = (
                        nxt[:, 0 : 2 * N]
                        .rearrange("p (b t f) -> p t b f", t=2, f=2)
                    )
                    nc.vector.tensor_scalar(
                        dst, src, 0.0, None, mybir.AluOpType.add
                    )
                else:
                    # final stage: quarter-size combines, each store chunk on
                    # a rotating queue, so output bytes leave early and the
                    # end-of-kernel tail is one small transfer. HWDGE queues
                    # only — SWDGE (gpsimd) descriptor generation contends
                    # with the DVE's shared port.
                    qfuncs = [nc.sync.dma_start]
                    if g == NGROUPS - 1:
                        qfuncs = [nc.sync.dma_start, nc.scalar.dma_start,
                                  nc.gpsimd.dma_start]
                    qi = 0
                    for hb in (0, 1):
                        for qq in (0, 1):
                            sb = hb * 2048 + qq * 1024
                            src = (
                                cur[:, 0 : 3 * N]
                                .rearrange("p (h f) -> p h f", h=3)[
                                    :, 0:3:2, sb : sb + 1024
                                ]
                            )
                            ob = obuf[:, hb * N + qq * 2048 :
                                      hb * N + qq * 2048 + 2048]
                            dst = ob.rearrange(
                                "p (b t f) -> p t b f", t=2, f=2
                            )
                            nc.vector.tensor_scalar(
                                dst, src, 0.0, None, mybir.AluOpType.add
                            )
                            off = hb * N + qq * 2048
                            for dh in (0, 1):
                                qfuncs[qi % len(qfuncs)](
                                    outT[rows, off + dh * 1024 :
                                         off + dh * 1024 + 1024],
                                    ob[:, dh * 1024 : dh * 1024 + 1024],
                                )
                                qi += 1

            # software pipeline: group g+1's ph/quads/copies are emitted
            # between stage 4 and 5 of group g (so the in-order vector queue
            # never blocks on a cross-engine copy or a late DMA), and x is
            # prefetched a full group earlier
            load_x(0)
            load_x(1)
            load_ph(0)
            quads_group(0)
            copies_group(0)
            sins_rest(0)
            for g in range(NGROUPS):
                obuf = opool.tile([128, 2 * N], F16, tag="o")
                for si, s in enumerate(range(3, 5)):
                    one_stage(g, si, s, obuf)
                if g + 2 < NGROUPS:
                    load_x(g + 2)
                if g + 1 < NGROUPS:
                    load_ph(g + 1)
                    quads_group(g + 1)
                    copies_group(g + 1)
                    sins_rest(g + 1)
                for si, s in enumerate(range(5, LOGN + 1), start=2):
                    one_stage(g, si, s, obuf)

    nc.compile()
    return nc


# ===================== host side ===========================================


def _low_perm_bits(s):
    bits = [1, 0]
    for t in range(3, s):
        bits.append(t - 1)
    return bits


def _pack_kidx(s):
    half = 1 << (s - 1)
    lowbits = _low_perm_bits(s)
    Ls = np.arange(half)
    gv = np.zeros(half, dtype=np.int64)
    for i, b in enumerate(lowbits):
        gv |= (((Ls >> (s - 2 - i)) & 1) << b)
    return gv * (N >> s)


def _topL_of_stage(s):
    cs = np.arange(K)
    lowbits = _low_perm_bits(s)
    nlow = s - 1
    nh = max(0, 11 - s)
    low = cs & ((1 << nlow) - 1)
    rest = cs >> nlow
    h = rest & ((1 << nh) - 1) if nh else np.zeros(K, dtype=np.int64)
    bs = rest >> nh
    j = np.zeros(K, dtype=np.int64)
    for i, b in enumerate(lowbits):
        j |= (((low >> (nlow - 1 - i)) & 1) << b)
    for i in range(nh):
        j |= (((h >> (nh - 1 - i)) & 1) << (s + 1 + i))
    if s <= 11:
        j |= bs << s
    return j


def _host_tables():
    # xr column order: [even-q (a,b) | odd-q (a,b) | even-q (c,d) | odd-q (c,d)]
    # quad rank r = [b3 | b4..b11 (b4 at bit 7 .. b11 at bit 0)]
    col = np.empty(N, dtype=np.int64)
    rr = np.arange(512)
    b3 = rr >> 8
    hv = rr & 0xFF  # bits: b4 at bit7 ... b11 at bit0
    q = np.zeros(512, dtype=np.int64)
    for i in range(8):
        q |= (((hv >> (7 - i)) & 1) << (2 + i))  # b_{4+i} -> quad bit 2+i
    q |= b3 << 1
    for par in (0, 1):
        qq = (q | par) << 2  # j = 4*quad
        base_ab = par * 1024
        base_cd = 2048 + par * 1024
        col[base_ab + 2 * rr] = qq + 0      # a
        col[base_ab + 2 * rr + 1] = qq + 1  # b
        col[base_cd + 2 * rr] = qq + 2      # c
        col[base_cd + 2 * rr + 1] = qq + 3  # d
    col = col ^ (N // 2)  # fold the reference's initial permutation

    # final output permutation: O granule p holds logical row fp[p]
    topL12 = _topL_of_stage(12)
    fp = np.empty(N, dtype=np.int64)
    fp[0::2] = topL12
    fp[1::2] = topL12 + 2048
    pos_of = np.empty(N, dtype=np.int64)
    pos_of[fp] = np.arange(N)
    return col, pos_of


_COL, _POS_OF = _host_tables()


def make_core_inputs(x: np.ndarray, weights: np.ndarray, core: int):
    sl = slice(core * DSH, (core + 1) * DSH)
    # xT: [DSH, N] fp16, columns = FFT points in quad layout order
    xT = np.ascontiguousarray(x[_COL][:, sl].T).astype(np.float16)

    # phase image per feature: el layout mirrors the on-chip pack
    w = weights[: N // 2, sl].astype(np.float64)  # [2048, DSH]
    k = -(1.0 / N) * np.arange(N // 2, dtype=np.float64)
    rrall = w * k[:, None]
    rrall -= np.rint(rrall)  # [2048, DSH] range-reduced sin phases
    ph = np.zeros((2 * N, DSH), dtype=np.float64)
    # stage-2 scalars at els 0..2: cos, sin, -sin phases of rr[1024]
    r2 = rrall[1024]
    ph[0] = 0.25 - np.abs(r2)
    ph[1] = r2
    ph[2] = -r2
    for s in range(3, LOGN + 1):
        kidx = _pack_kidx(s)  # [half]
        rs = rrall[kidx]  # [half, DSH]
        base = 1 << s
        ph[base + 0 : base + 2 * len(kidx) : 2] = 0.25 - np.abs(rs)  # cos
        ph[base + 1 : base + 2 * len(kidx) : 2] = rs  # sin
    phT = np.ascontiguousarray(ph.T).astype(np.float16)
    return {"xT": xT, "phT": phT}


def assemble_output(core_outs):
    full = np.empty((N, N), dtype=np.complex64)
    for c, r in enumerate(core_outs):
        oc = r["outT"].astype(np.float32).view(np.complex64)  # [DSH, N]
        full[:, c * DSH : (c + 1) * DSH] = oc[:, _POS_OF].T
    return full


_NC_CACHE = None


def get_nc():
    global _NC_CACHE
    if _NC_CACHE is None:
        _NC_CACHE = build_fft_nc()
    return _NC_CACHE


def run_on_hw(x, weights, **spmd_kwargs):
    nc = get_nc()
    x = np.asarray(x, dtype=np.float32)
    weights = np.asarray(weights, dtype=np.float32)
    in_maps = [make_core_inputs(x, weights, c) for c in range(NCORES)]
    res = run_bass_kernel_spmd(nc, in_maps, core_ids=list(range(NCORES)),
                               **spmd_kwargs)
    return assemble_output(res.results), res


def kernel(x: np.ndarray, weights: np.ndarray) -> np.ndarray:
    out, _ = run_on_hw(x, weights)
    return out



# revision 26
# speedup vs baseline: 1.0122x; 1.0122x over previous
"""Trainium2 Bass kernel v3: constant-geometry data-dependent FFT.

Structure per core (512 features = 4 groups x 128 partitions; features on
partitions, FFT points along the free dim, packed complex fp16 granules):

  - Host sends x fp16 with rows pre-permuted into quad layout, and a
    phase image PH holding every twiddle phase (range-reduced, incl. the
    cos phases as 0.25-|r|) laid out exactly like the on-chip pack.
  - ONE Sin activation turns PH into the complete twiddle pack (fp16
    (cos,sin) granules, stage s at el offset 2^s).
  - Stages 1+2: 8 custom 2X DVE ops over row quads -> T3/B3.
  - Stages 3..12: per stage:
      CMUL (custom 2X, 2-tensor):  U = pack_s (*) B
      combineT/combineB: hijacked stock TENSOR_SCALAR row running a
      4X_2PORT butterfly uop program (2 granules/cycle): reads the
      (T half || U half) 2-run AP, writes y0/y1 interleaved into the
      next stage's T (or B) region.
  - Output O (fp16 packed complex) DMA'd out in 2 chunks; host converts
    to complex64 and un-permutes rows (bit-reversal-like final_perm).

Everything validated against the reference network in sim_plan3.py (exact)
and perf modes/port semantics probed on hardware (bench_dve/bench_hijack*).
"""

import math
import sys

import numpy as np

if "/opt/trn_rl_repo" not in sys.path:
    sys.path.insert(0, "/opt/trn_rl_repo")

import concourse.bacc as bacc
import concourse.bass_isa as bass_isa
import concourse.mybir as mybir
from concourse.bass_utils import run_bass_kernel_spmd
from concourse.tile import TileContext

import concourse.dve_ops as dve_ops
from concourse.dve_spec import Spec, Src0, Src1
from concourse.dve_uop import (
    AluInp,
    AluOp,
    DelayInp,
    DveOpSpec,
    InpSel,
    OutPath,
    OutSel,
    Trigger,
    UopConfig,
)

F32 = mybir.dt.float32
F16 = mybir.dt.float16
AF = mybir.ActivationFunctionType

N = 4096
LOGN = 12
K = N // 2  # butterflies per stage
NCORES = 8
DSH = N // NCORES
NGROUPS = DSH // 128
TWO_PI = 2.0 * math.pi

D = [
    AluInp.PREV_DELAY_0,
    AluInp.PREV_DELAY_1,
    AluInp.PREV_DELAY_2,
    AluInp.PREV_DELAY_3,
    AluInp.PREV_DELAY_4,
    AluInp.PREV_DELAY_5,
]


def _uop(inputs, req0, req1, trigger, next_uop, repeat=0):
    u = UopConfig()
    for lane, sel in enumerate(inputs, start=1):
        u.enable_input(sel, lane)
    u.require_inp0 = req0
    u.require_inp1 = req1
    u.trigger = trigger
    u.next_uop = next_uop
    u.repeat_count = repeat
    return u


_TRIG1 = (Trigger.SRC_TENSOR_DONE, Trigger.NONE, Trigger.NONE)


# ===================== custom 2X ops (CMUL + stage-1/2 quads) ==============


def _cmul_uop():
    # packed complex multiply: out = in0 (*) in1, 1 granule/cycle in 2X
    u = _uop(
        [InpSel.SRC_0, InpSel.SRC_1, InpSel.SRC_0_HI, InpSel.SRC_1_HI],
        1, 1, _TRIG1, (0, 0, 0),
    )
    dp = u.datapath_config
    dp[0].enable_alu(AluOp.MULTIPLY, D[0], D[1])
    dp[0].pass_through_delay(0, 1, 2, 3)
    dp[1].enable_alu(AluOp.MULTIPLY, D[2], D[3])
    dp[1].pass_through_delay(0, 1, 2, 3)
    dp[1].enable_delay_from_src(DelayInp.PREV_ALU_OUT, 4)
    dp[2].enable_alu(AluOp.SUBTRACT, D[4], AluInp.PREV_ALU_OUT)
    dp[2].pass_through_delay(0, 1, 2, 3)
    dp[3].enable_alu(AluOp.MULTIPLY, D[0], D[3])
    dp[3].pass_through_delay(1, 2)
    dp[3].enable_delay_from_src(DelayInp.PREV_ALU_OUT, 4)
    dp[4].enable_alu(AluOp.MULTIPLY, D[2], D[1])
    dp[4].pass_through_delay(4)
    dp[4].enable_delay_from_src(DelayInp.PREV_ALU_OUT, 0)
    dp[5].enable_alu(AluOp.ADD, D[0], AluInp.PREV_ALU_OUT)
    dp[5].pass_through_delay(4)
    dp[6].pass_through_alu()
    dp[6].pass_through_delay(4)
    dp[7].pass_through_alu()
    dp[7].pass_through_delay(4)
    u.enable_output(OutSel.DELAY_4, OutPath.WR0_LO)
    u.enable_output(OutSel.ALU_OUT, OutPath.WR0_HI)
    return u


def _cmul_reference(in0, in1, c0, c1, c2):
    a = in0.astype(np.float32)
    b = np.broadcast_to(in1, in0.shape).astype(np.float32)
    out = np.empty_like(a)
    ar, ai = a[..., 0::2], a[..., 1::2]
    br, bi = b[..., 0::2], b[..., 1::2]
    out[..., 0::2] = ar * br - ai * bi
    out[..., 1::2] = ar * bi + ai * br
    return out


# stage-1/2 quad ops: per cycle granule A=(a,b), granule B=(c,d); one
# output granule. t0=a+b t1=c+d t2=a-b u=c-d.
#  S3K0 -> (t0+t1, 0)      S3K2 -> (t0-t1, 0)
#  S3K1 -> (t2 + C0*u, C1*u)   [s0=cos col, s1=sin col]
#  S3K3 -> (t2 - C0*u, C1*u)   [s0=cos col, s1=-sin col]


def _s3k_sum(sub: bool):
    u = _uop(
        [InpSel.SRC_0, InpSel.SRC_0_HI, InpSel.SRC_1, InpSel.SRC_1_HI,
         InpSel.ZERO],
        1, 1, _TRIG1, (0, 0, 0),
    )
    dp = u.datapath_config
    dp[0].enable_alu(AluOp.ADD, D[0], D[1])  # t0
    dp[0].pass_through_delay(2, 3, 4)
    dp[1].enable_alu(AluOp.ADD, D[2], D[3])  # t1
    dp[1].enable_delay_from_src(DelayInp.PREV_ALU_OUT, 0)  # t0
    dp[1].pass_through_delay(2, 3, 4)
    op = AluOp.SUBTRACT if sub else AluOp.ADD
    dp[2].enable_alu(op, D[0], AluInp.PREV_ALU_OUT)  # re = t0 +/- t1
    dp[2].pass_through_delay(4)
    for k in range(3, 8):
        dp[k].pass_through_alu()
        dp[k].pass_through_delay(4)
    u.enable_output(OutSel.ALU_OUT, OutPath.WR0_LO)  # re
    u.enable_output(OutSel.DELAY_4, OutPath.WR0_HI)  # 0
    return u


def _s3k_tw(sub: bool):
    u = _uop(
        [InpSel.SRC_0, InpSel.SRC_0_HI, InpSel.SRC_1, InpSel.SRC_1_HI,
         InpSel.CONST_0, InpSel.CONST_1],
        1, 1, _TRIG1, (0, 0, 0),
    )
    dp = u.datapath_config
    dp[0].enable_alu(AluOp.SUBTRACT, D[0], D[1])  # t2 = a-b
    dp[0].pass_through_delay(2, 3, 4, 5)
    dp[1].enable_alu(AluOp.SUBTRACT, D[2], D[3])  # u = c-d
    dp[1].enable_delay_from_src(DelayInp.PREV_ALU_OUT, 0)  # t2
    dp[1].pass_through_delay(4, 5)
    dp[2].enable_alu(AluOp.MULTIPLY, AluInp.PREV_ALU_OUT, D[4])  # mc = C0*u
    dp[2].enable_delay_from_src(DelayInp.PREV_ALU_OUT, 1)  # u
    dp[2].pass_through_delay(0, 5)
    op = AluOp.SUBTRACT if sub else AluOp.ADD
    dp[3].enable_alu(op, D[0], AluInp.PREV_ALU_OUT)  # re = t2 +/- mc
    dp[3].pass_through_delay(1, 5)
    dp[4].enable_alu(AluOp.MULTIPLY, D[1], D[5])  # im = C1*u
    dp[4].enable_delay_from_src(DelayInp.PREV_ALU_OUT, 0)  # re
    for k in range(5, 8):
        dp[k].pass_through_alu()
        dp[k].pass_through_delay(0)
    u.enable_output(OutSel.DELAY_0, OutPath.WR0_LO)  # re
    u.enable_output(OutSel.ALU_OUT, OutPath.WR0_HI)  # im
    return u


def _s3k_sum_ref(sub):
    def f(in0, in1, c0, c1, c2):
        a = in0.astype(np.float32)
        b = np.broadcast_to(in1, in0.shape).astype(np.float32)
        t0 = a[..., 0::2] + a[..., 1::2]
        t1 = b[..., 0::2] + b[..., 1::2]
        out = np.empty_like(a)
        out[..., 0::2] = t0 - t1 if sub else t0 + t1
        out[..., 1::2] = 0.0
        return out

    return f


def _s3k_tw_ref(sub):
    def f(in0, in1, c0, c1, c2):
        a = in0.astype(np.float32)
        b = np.broadcast_to(in1, in0.shape).astype(np.float32)
        c0 = np.asarray(c0, np.float32).reshape(-1, *([1] * (a.ndim - 1)))
        c1 = np.asarray(c1, np.float32).reshape(-1, *([1] * (a.ndim - 1)))
        t2 = a[..., 0::2] - a[..., 1::2]
        u = b[..., 0::2] - b[..., 1::2]
        out = np.empty_like(a)
        out[..., 0::2] = t2 - c0 * u if sub else t2 + c0 * u
        out[..., 1::2] = c1 * u
        return out

    return f


# ===================== 4X combine on the hijacked stock row ================

TS_ROW_OPCODES = (67, 68)  # TENSOR_SCALAR family rows (imm + ptr variants)


def _bfly4x():
    # half-split ports: A granule from in-AP first half, B from second half;
    # WR0 (first half of out walk) = A+B, WR1 (second half) = A-B.
    u = _uop(
        [InpSel.SRC_0, InpSel.SRC_0_HI, InpSel.SRC_1, InpSel.SRC_1_HI],
        1, 1, _TRIG1, (0, 0, 0),
    )
    u.enable_input(InpSel.SRC_0, 0)
    dp = u.datapath_config
    dp[0].enable_alu(AluOp.ADD, D[0], D[2])
    dp[0].pass_through_delay(0, 1, 2, 3)
    dp[1].enable_alu(AluOp.ADD, D[1], D[3])
    dp[1].enable_delay_from_src(DelayInp.PREV_ALU_OUT, 4)
    dp[1].pass_through_delay(0, 1, 2, 3)
    dp[2].enable_alu(AluOp.SUBTRACT, D[0], D[2])
    dp[2].enable_delay_from_src(DelayInp.PREV_ALU_OUT, 5)
    dp[2].pass_through_delay(1, 3, 4)
    dp[3].enable_alu(AluOp.SUBTRACT, D[1], D[3])
    dp[3].enable_delay_from_src(DelayInp.PREV_ALU_OUT, 0)
    dp[3].pass_through_delay(4, 5)
    for k in range(4, 8):
        dp[k].pass_through_alu()
        dp[k].pass_through_delay(0, 4, 5)
    u.enable_output(OutSel.DELAY_4, OutPath.WR0_LO)
    u.enable_output(OutSel.DELAY_5, OutPath.WR0_HI)
    u.enable_output(OutSel.DELAY_0, OutPath.WR1_LO)
    u.enable_output(OutSel.ALU_OUT, OutPath.WR1_HI)
    return u


def _bfly_ref(in0, in1, c0, c1, c2):
    return in0.astype(np.float32)


# ===================== registry ============================================


class StockRowSpec(DveOpSpec):
    """Spec for a hijacked STOCK row — skip the custom_dve-only uop guards."""

    def validate(self, ver):
        for uops in (self.uops, self.uops_2x, self.uops_2x_2p, self.uops_4x):
            if uops is None:
                continue
            for u in uops:
                for ni in u.next_uop:
                    assert ni < len(uops)


class RawDveOp:
    def __init__(self, name, mk_uops, rd1_en, perf_max, reference,
                 opcode=None, all_modes=False):
        self.name = name
        self.subdim = False
        self.spec = Spec(body=Src0 * Src1 if rd1_en else Src0,
                         reference=reference)
        self.rd1_en = rd1_en
        self.perf_max = perf_max
        self.opcode = opcode
        self.all_modes = all_modes
        self._mk = mk_uops
        self._cache = {}

    def compile(self, ver):
        if ver in self._cache:
            return self._cache[ver]
        cls = StockRowSpec if self.opcode is not None else DveOpSpec
        spec = cls(
            name=self.name,
            uops=[self._mk()],
            opcode=self.opcode
            if self.opcode is not None
            else dve_ops.get_dve_sub_opcode(self.name),
            uops_2x=[self._mk()],
            uops_2x_2p=[self._mk()] if self.all_modes else None,
            uops_4x=[self._mk()] if self.all_modes else None,
            perf_max=self.perf_max,
            rd1_en=self.rd1_en,
        )
        spec.validate(ver)
        self._cache[ver] = spec
        return spec


RAW3 = {}


def register_raw_ops3():
    if RAW3:
        return RAW3
    defs = [
        RawDveOp("CMUL3_ANT", _cmul_uop, True, 1, _cmul_reference),
        RawDveOp("S3K0_ANT", lambda: _s3k_sum(False), True, 1, _s3k_sum_ref(False)),
        RawDveOp("S3K2_ANT", lambda: _s3k_sum(True), True, 1, _s3k_sum_ref(True)),
        RawDveOp("S3K1_ANT", lambda: _s3k_tw(False), True, 1, _s3k_tw_ref(False)),
        RawDveOp("S3K3_ANT", lambda: _s3k_tw(True), True, 1, _s3k_tw_ref(True)),
        RawDveOp("BFLY43_HJ", _bfly4x, False, 0, _bfly_ref, opcode=67,
                 all_modes=True),
        RawDveOp("BFLY44_HJ", _bfly4x, False, 0, _bfly_ref, opcode=68,
                 all_modes=True),
    ]
    for op in defs:
        if op.name not in dve_ops._SUB_OPCODE_FOR_NAME:
            dve_ops.OPS.append(op)
            if op.opcode is None:
                row = dve_ops._CUSTOM_DVE_ROW_BASE + len(dve_ops.OPS) - 1
                assert row < 0x20, row
                dve_ops._SUB_OPCODE_FOR_NAME[op.name] = row
            else:
                dve_ops._SUB_OPCODE_FOR_NAME[op.name] = op.opcode
            dve_ops.CUSTOM_DVE_SPECS[op.name] = op.spec
        RAW3[op.name] = op
    return RAW3


def emit_raw3(nc, name, out, in0, in1=None, s0=None, s1=None, perf_max=1,
              force_ttss=False):
    ops = register_raw_ops3()
    op = ops[name]
    v = nc.vector
    if op.name not in nc.m.ant_custom_dve_ops:
        nc.m.ant_custom_dve_ops = sorted({*nc.m.ant_custom_dve_ops, op.name})
    use_stt = in1 is not None and not force_ttss
    shape = bass_isa.CustomDveShape.STT if use_stt else bass_isa.CustomDveShape.TTSS
    isa_opcode = nc.isa.Opcode[
        f"NEURON_ISA_TPB_OPCODE_CUSTOM_DVE_ANT_{shape.slot()}"
    ].value
    imm = mybir.ImmediateValue(dtype=mybir.dt.float32, value=0.0)
    s0a = v.lower_ap(s0, for_isa=True) if s0 is not None else imm
    s1a = v.lower_ap(s1, for_isa=True) if s1 is not None else imm
    ins = [v.lower_ap(in0, for_isa=True)]
    if in1 is not None:
        ins.append(v.lower_ap(in1, for_isa=True))
    ins += [s0a, s1a]
    return v.add_instruction(
        bass_isa.InstCustomDveAnt(
            name=nc.get_next_instruction_name(),
            op_name=op.name,
            rd1_en=op.rd1_en,
            subdim=0,
            imm2=0.0,
            shape=shape,
            row=dve_ops.get_dve_sub_opcode(op.name),
            isa_opcode=isa_opcode,
            perf_max=perf_max,
            ins=ins,
            outs=[v.lower_ap(out, for_isa=True)],
        )
    )


# ===================== kernel builder ======================================


def build_fft_nc():
    register_raw_ops3()
    nc = bacc.Bacc()
    # hijacked stock rows must land in the per-NEFF DVE table
    nc.m.ant_custom_dve_ops = sorted(
        {*nc.m.ant_custom_dve_ops, "BFLY43_HJ", "BFLY44_HJ"}
    )

    xT = nc.dram_tensor("xT", [DSH, N], F16, kind="ExternalInput")
    phT = nc.dram_tensor("phT", [DSH, 2 * N], F16, kind="ExternalInput")
    outT = nc.dram_tensor("outT", [DSH, 2 * N], F16, kind="ExternalOutput")

    EL_T = 0          # T region: els [0, 4096)
    EL_B = N          # B region: els [4096, 8192)  (adjacent to T!)
    EL_U = 2 * N      # U region: els [8192, 12288)

    with TileContext(nc) as tc:
        with (
            tc.tile_pool(name="xin", bufs=2) as xpool,
            tc.tile_pool(name="pk", bufs=2) as kpool,
            tc.tile_pool(name="tub", bufs=2) as tpool,
            tc.tile_pool(name="stg", bufs=2) as spool,
            tc.tile_pool(name="obuf", bufs=1) as opool,
        ):
            packs = {}
            c2s = {}
            xrs = {}
            stags = {}
            tubss = {}

            def load_x(g):
                # Queue discipline: gpsimd carries only tiny latency-critical
                # transfers (ph heads, group-0 x halves); the scalar HWDGE
                # queue carries dependency-free bulk input (its triggers fire
                # immediately, so they never block later scalar-engine work);
                # sync carries group-0 x halves and all output stores.
                rows = slice(g * 128, (g + 1) * 128)
                xr = xpool.tile([128, N], F16, tag="xr")
                xrs[g] = xr
                if g == 0:
                    # group 0's x is latency-critical: stripe the four
                    # quarter-chunks across the two HWDGE queues (both idle
                    # at t=0), par-0 cols first
                    nc.sync.dma_start(xr[:, 0:1024], xT[rows, 0:1024])
                    nc.scalar.dma_start(xr[:, 2048:3072], xT[rows, 2048:3072])
                    nc.sync.dma_start(xr[:, 1024:2048], xT[rows, 1024:2048])
                    nc.scalar.dma_start(xr[:, 3072:4096], xT[rows, 3072:4096])
                else:
                    nc.scalar.dma_start(xr[:], xT[rows, :])

            def load_ph(g):
                rows = slice(g * 128, (g + 1) * 128)
                # pack doubles as the ph landing buffer: Sin runs in place.
                pack = kpool.tile([128, 2 * N], F16, tag="pack")
                c2 = kpool.tile([128, 4], F32, tag="c2")
                packs[g], c2s[g] = pack, c2
                # ph head on gpsimd: lands fast, c2 is on the quads' critical
                # path. mid covers stage 7..9 twiddles, rest stages 10..12.
                nc.gpsimd.dma_start(pack[:, 0:128], phT[rows, 0:128])
                nc.scalar.dma_start(pack[:, 128:2048], phT[rows, 128:2048])
                nc.scalar.dma_start(pack[:, 2048 : 2 * N], phT[rows, 2048 : 2 * N])
                nc.scalar.activation(c2[:, 0:3], pack[:, 0:3], AF.Sin,
                                     scale=TWO_PI)
                nc.scalar.activation(pack[:, 0:128], pack[:, 0:128], AF.Sin,
                                     scale=TWO_PI)

            def sins_rest(g):
                pack = packs[g]
                nc.scalar.activation(pack[:, 128:2048], pack[:, 128:2048],
                                     AF.Sin, scale=TWO_PI)
                nc.scalar.activation(pack[:, 2048 : 2 * N], pack[:, 2048 : 2 * N],
                                     AF.Sin, scale=TWO_PI)

            def quads_group(g):
                # stages 1+2: 8 quad ops (2X, contiguous outs into staging)
                xr, c2 = xrs[g], c2s[g]
                stag = spool.tile([128, 2 * N], F16, tag="stg")
                stags[g] = stag
                tubA = tpool.tile([128, 3 * N], F16, tag="tubA")
                tubB = tpool.tile([128, 3 * N], F16, tag="tubB")
                tubss[g] = [tubA, tubB]
                for par in (0, 1):
                    ab = xr[:, par * 1024 : par * 1024 + 1024]
                    cd = xr[:, 2048 + par * 1024 : 2048 + par * 1024 + 1024]
                    sb = par * N  # staging region per parity: k-major blocks
                    emit_raw3(nc, "S3K0_ANT",
                              stag[:, sb + 0 : sb + 1024], ab, cd)
                    emit_raw3(nc, "S3K1_ANT",
                              stag[:, sb + 1024 : sb + 2048], ab, cd,
                              s0=c2[:, 0:1], s1=c2[:, 1:2], force_ttss=True)
                    emit_raw3(nc, "S3K2_ANT",
                              stag[:, sb + 2048 : sb + 3072], ab, cd)
                    emit_raw3(nc, "S3K3_ANT",
                              stag[:, sb + 3072 : sb + 4096], ab, cd,
                              s0=c2[:, 0:1], s1=c2[:, 2:3], force_ttss=True)

            def copies_group(g):
                # scatter quad-order staging into pair-order T3/B3 on the
                # (otherwise idle) scalar engine; dst granule 4r+k <- src
                # granule k*512+r
                stag, cur = stags[g], tubss[g][0]
                for par in (0, 1):
                    sb = par * N
                    base = EL_T if par == 0 else EL_B
                    r4 = (
                        cur[:, base : base + N]
                        .rearrange("p (r k f) -> p k r f", k=4, f=2)
                    )
                    # group 0's copies are on the critical path (nothing else
                    # for the DVE to chew on), so the DVE itself scatters the
                    # par-1 (B3) half it needs first, while ACT does par-0
                    eng = nc.vector if (g == 0 and par == 1) else nc.scalar
                    if eng is nc.vector:
                        eng.tensor_copy(r4[:, 0:2], stag[:, sb : sb + 2048])
                        eng.tensor_copy(r4[:, 2:4], stag[:, sb + 2048 : sb + 4096])
                    else:
                        eng.copy(r4[:, 0:2], stag[:, sb : sb + 2048])
                        eng.copy(r4[:, 2:4], stag[:, sb + 2048 : sb + 4096])

            def one_stage(g, si, s, obuf):
                rows = slice(g * 128, (g + 1) * 128)
                pack, tubs = packs[g], tubss[g]
                cur = tubs[si % 2]
                nxt = tubs[(si + 1) % 2]
                half = 1 << (s - 1)
                rep = N >> s
                twp = pack[:, 2 * half : 4 * half]
                if rep > 1:
                    tw = twp.unsqueeze(1).broadcast_to([128, rep, 2 * half])
                else:
                    tw = twp
                emit_raw3(
                    nc, "CMUL3_ANT",
                    cur[:, EL_U : EL_U + N],
                    cur[:, EL_B : EL_B + N],
                    tw,
                )
                if s < LOGN:
                    # one combine: in = (all T || all U) 2-run AP; out =
                    # y0/y1 interleaved over the adjacent T|B dest region
                    src = (
                        cur[:, 0 : 3 * N]
                        .rearrange("p (h f) -> p h f", h=3)[:, 0:3:2]
                    )
                    dst = (
                        nxt[:, 0 : 2 * N]
                        .rearrange("p (b t f) -> p t b f", t=2, f=2)
                    )
                    nc.vector.tensor_scalar(
                        dst, src, 0.0, None, mybir.AluOpType.add
                    )
                else:
                    # final stage: small combines, each store chunk on a
                    # rotating queue, so output bytes leave early and the
                    # end-of-kernel tail is one small transfer. HWDGE queues
                    # only — SWDGE (gpsimd) descriptor generation contends
                    # with the DVE's shared port.
                    last = g == NGROUPS - 1
                    qfuncs = [nc.sync.dma_start]
                    if last:
                        qfuncs = [nc.sync.dma_start, nc.scalar.dma_start]
                    qi = 0
                    nq = 4 if last else 2  # combine slices per hb half
                    for hb in (0, 1):
                        for qq in range(nq):
                            w = 2048 // nq  # src els per run in this slice
                            sb = hb * 2048 + qq * w
                            src = (
                                cur[:, 0 : 3 * N]
                                .rearrange("p (h f) -> p h f", h=3)[
                                    :, 0:3:2, sb : sb + w
                                ]
                            )
                            off = hb * N + qq * 2 * w
                            ob = obuf[:, off : off + 2 * w]
                            dst = ob.rearrange(
                                "p (b t f) -> p t b f", t=2, f=2
                            )
                            nc.vector.tensor_scalar(
                                dst, src, 0.0, None, mybir.AluOpType.add
                            )
                            for dh in (0, 1):
                                qfuncs[qi % len(qfuncs)](
                                    outT[rows, off + dh * w : off + dh * w + w],
                                    ob[:, dh * w : dh * w + w],
                                )
                                qi += 1

            # software pipeline: group g+1's ph/quads/copies are emitted
            # between stage 4 and 5 of group g (so the in-order vector queue
            # never blocks on a cross-engine copy or a late DMA), and x is
            # prefetched a full group earlier
            load_x(0)
            load_x(1)
            load_ph(0)
            quads_group(0)
            copies_group(0)
            sins_rest(0)
            for g in range(NGROUPS):
                obuf = opool.tile([128, 2 * N], F16, tag="o")
                for si, s in enumerate(range(3, 5)):
                    one_stage(g, si, s, obuf)
                if g + 2 < NGROUPS:
                    load_x(g + 2)
                if g + 1 < NGROUPS:
                    load_ph(g + 1)
                    quads_group(g + 1)
                    copies_group(g + 1)
                    sins_rest(g + 1)
                for si, s in enumerate(range(5, LOGN + 1), start=2):
                    one_stage(g, si, s, obuf)

    nc.compile()
    return nc


# ===================== host side ===========================================


def _low_perm_bits(s):
    bits = [1, 0]
    for t in range(3, s):
        bits.append(t - 1)
    return bits


def _pack_kidx(s):
    half = 1 << (s - 1)
    lowbits = _low_perm_bits(s)
    Ls = np.arange(half)
    gv = np.zeros(half, dtype=np.int64)
    for i, b in enumerate(lowbits):
        gv |= (((Ls >> (s - 2 - i)) & 1) << b)
    return gv * (N >> s)


def _topL_of_stage(s):
    cs = np.arange(K)
    lowbits = _low_perm_bits(s)
    nlow = s - 1
    nh = max(0, 11 - s)
    low = cs & ((1 << nlow) - 1)
    rest = cs >> nlow
    h = rest & ((1 << nh) - 1) if nh else np.zeros(K, dtype=np.int64)
    bs = rest >> nh
    j = np.zeros(K, dtype=np.int64)
    for i, b in enumerate(lowbits):
        j |= (((low >> (nlow - 1 - i)) & 1) << b)
    for i in range(nh):
        j |= (((h >> (nh - 1 - i)) & 1) << (s + 1 + i))
    if s <= 11:
        j |= bs << s
    return j


def _host_tables():
    # xr column order: [even-q (a,b) | odd-q (a,b) | even-q (c,d) | odd-q (c,d)]
    # quad rank r = [b3 | b4..b11 (b4 at bit 7 .. b11 at bit 0)]
    col = np.empty(N, dtype=np.int64)
    rr = np.arange(512)
    b3 = rr >> 8
    hv = rr & 0xFF  # bits: b4 at bit7 ... b11 at bit0
    q = np.zeros(512, dtype=np.int64)
    for i in range(8):
        q |= (((hv >> (7 - i)) & 1) << (2 + i))  # b_{4+i} -> quad bit 2+i
    q |= b3 << 1
    for par in (0, 1):
        qq = (q | par) << 2  # j = 4*quad
        base_ab = par * 1024
        base_cd = 2048 + par * 1024
        col[base_ab + 2 * rr] = qq + 0      # a
        col[base_ab + 2 * rr + 1] = qq + 1  # b
        col[base_cd + 2 * rr] = qq + 2      # c
        col[base_cd + 2 * rr + 1] = qq + 3  # d
    col = col ^ (N // 2)  # fold the reference's initial permutation

    # final output permutation: O granule p holds logical row fp[p]
    topL12 = _topL_of_stage(12)
    fp = np.empty(N, dtype=np.int64)
    fp[0::2] = topL12
    fp[1::2] = topL12 + 2048
    pos_of = np.empty(N, dtype=np.int64)
    pos_of[fp] = np.arange(N)
    return col, pos_of


_COL, _POS_OF = _host_tables()


def make_core_inputs(x: np.ndarray, weights: np.ndarray, core: int):
    sl = slice(core * DSH, (core + 1) * DSH)
    # xT: [DSH, N] fp16, columns = FFT points in quad layout order
    xT = np.ascontiguousarray(x[_COL][:, sl].T).astype(np.float16)

    # phase image per feature: el layout mirrors the on-chip pack
    w = weights[: N // 2, sl].astype(np.float64)  # [2048, DSH]
    k = -(1.0 / N) * np.arange(N // 2, dtype=np.float64)
    rrall = w * k[:, None]
    rrall -= np.rint(rrall)  # [2048, DSH] range-reduced sin phases
    ph = np.zeros((2 * N, DSH), dtype=np.float64)
    # stage-2 scalars at els 0..2: cos, sin, -sin phases of rr[1024]
    r2 = rrall[1024]
    ph[0] = 0.25 - np.abs(r2)
    ph[1] = r2
    ph[2] = -r2
    for s in range(3, LOGN + 1):
        kidx = _pack_kidx(s)  # [half]
        rs = rrall[kidx]  # [half, DSH]
        base = 1 << s
        ph[base + 0 : base + 2 * len(kidx) : 2] = 0.25 - np.abs(rs)  # cos
        ph[base + 1 : base + 2 * len(kidx) : 2] = rs  # sin
    phT = np.ascontiguousarray(ph.T).astype(np.float16)
    return {"xT": xT, "phT": phT}


def assemble_output(core_outs):
    full = np.empty((N, N), dtype=np.complex64)
    for c, r in enumerate(core_outs):
        oc = r["outT"].astype(np.float32).view(np.complex64)  # [DSH, N]
        full[:, c * DSH : (c + 1) * DSH] = oc[:, _POS_OF].T
    return full


_NC_CACHE = None


def get_nc():
    global _NC_CACHE
    if _NC_CACHE is None:
        _NC_CACHE = build_fft_nc()
    return _NC_CACHE


def run_on_hw(x, weights, **spmd_kwargs):
    nc = get_nc()
    x = np.asarray(x, dtype=np.float32)
    weights = np.asarray(weights, dtype=np.float32)
    in_maps = [make_core_inputs(x, weights, c) for c in range(NCORES)]
    res = run_bass_kernel_spmd(nc, in_maps, core_ids=list(range(NCORES)),
                               **spmd_kwargs)
    return assemble_output(res.results), res


def kernel(x: np.ndarray, weights: np.ndarray) -> np.ndarray:
    out, _ = run_on_hw(x, weights)
    return out

